# revision 21
# baseline (speedup 1.0000x reference)
"""MoE (8 experts, top-2, d=1024, N=8192) on 8 trn2 NeuronCores.

Strategy (expert-parallel, per sharding hint):
 - Host computes routing + top-2 softmax gates in fp64 and dispatches tokens
   with the gate FOLDED INTO x (xg = g * x[idx], bf16).
 - Load balancing: instead of one expert per core (padded to the max expert
   load), each core holds NSLOT=3 expert-weight slots with a static per-tile
   schedule (6,6,5 tiles -> T=17 tiles/core).  The host bin-packs each
   expert's 128-token tiles into the 24 (core,slot) bins, so every core does
   the same (minimal) amount of matmul work regardless of routing skew.
 - Device (per core, SPMD): pure expert matmul y = xg @ W[slot] with PSUM
   K-accumulation in bf16; evict PSUM->SBUF as bf16 via alternating
   scalar/vector copies; DMA out.
 - Host combines: out[idx] += y + g*b[e].
"""

import os
from contextlib import ExitStack

import ml_dtypes
import numpy as np

import concourse.bass as bass
import concourse.bacc as bacc
import concourse.mybir as mybir
import concourse.tile as tile
from concourse.bass import ts
from concourse.bass_utils import run_bass_kernel_spmd

N_EXPERTS = 8
TOP_K = 2
D = 1024
N_CORES = 8
P = 128  # partitions
KT = D // P  # number of K tiles (8)
NH = 512  # psum free-dim per matmul (ISA max 512)

SLOT_CAPS = (6, 6, 5)  # tiles per weight slot per core (sum = T = 17)
UNROLL = int(os.environ.get("MOE_UNROLL", "2"))  # passes per hw-loop iteration

LAST_RESULTS = None  # stash of BassKernelResults for test harness inspection

_BUILD_CACHE = {}


def _build(caps, repeat=1):
    """Build the SPMD Bass module for per-core slot schedule `caps`
    (tuple of per-slot tile counts; one expert-weight slot each)."""
    key = (caps, repeat, os.environ.get("MOE_SR", "1"), UNROLL)
    if key in _BUILD_CACHE:
        return _BUILD_CACHE[key]

    f32 = mybir.dt.float32
    bf16 = mybir.dt.bfloat16
    T = sum(caps)
    S = len(caps)
    slot_of = []
    for s, cap in enumerate(caps):
        slot_of += [s] * cap

    nc = bacc.Bacc(None, target_bir_lowering=False)
    # xg_t: tiled gate-folded tokens [128 din-sub, T, KT, 128 tok]
    xg_t = nc.declare_dram_parameter("xg_t", [P, T * KT * P], bf16, isOutput=False)
    # stacked per-slot expert weights [S*D, D]
    w = nc.declare_dram_parameter("w", [S * D, D], bf16, isOutput=False)
    # output y tiled [128 tok-in-tile, T, 1024 feat]
    y = nc.declare_dram_parameter("y", [P, T * D], bf16, isOutput=True)

    with tile.TileContext(nc) as tc, ExitStack() as ctx:
        consts = ctx.enter_context(tc.tile_pool(name="consts", bufs=1))
        xpool = ctx.enter_context(tc.tile_pool(name="x", bufs=4))
        ypsum = ctx.enter_context(
            tc.tile_pool(name="ypsum", bufs=3, space=bass.MemorySpace.PSUM)
        )
        ypool = ctx.enter_context(tc.tile_pool(name="y", bufs=3))

        # ---- expert weights resident in SBUF: w_sb[p, s, kt, n] ----
        w_sb = consts.tile([P, S, KT, D], bf16)
        nc.sync.dma_start(w_sb[:], w.rearrange("(s kt p) n -> p s kt n", p=P, kt=KT))

        SR = os.environ.get("MOE_SR", "1") == "1"
        rep_cm = (
            tc.For_i(0, repeat, 1, staggered_reset=SR) if repeat > 1 else None
        )
        if rep_cm is not None:
            rep_cm.__enter__()
        unroll = UNROLL if repeat > 1 else 1

        for _ in range(unroll):
            for t in range(T):
                xt = xpool.tile([P, KT * P], bf16, tag="xt")
                nc.sync.dma_start(xt[:], xg_t[:, t * KT * P : (t + 1) * KT * P])
                yp = ypsum.tile([P, D], f32, tag="yp")
                for nh in range(D // NH):
                    for j in range(KT):
                        nc.tensor.matmul(
                            yp[:, ts(nh, NH)],
                            xt[:, ts(j, P)],
                            w_sb[:, slot_of[t], j, ts(nh, NH)],
                            start=(j == 0),
                            stop=(j == KT - 1),
                        )
                ysb = ypool.tile([P, D], bf16, tag="ysb")
                if t % 2 == 0:
                    nc.scalar.copy(ysb[:], yp[:])
                else:
                    nc.vector.tensor_copy(ysb[:], yp[:])
                nc.sync.dma_start(y[:, t * D : (t + 1) * D], ysb[:])

        if rep_cm is not None:
            rep_cm.__exit__(None, None, None)

    nc.compile()
    _BUILD_CACHE[key] = nc
    return nc


def _route(x, Wr, br):
    """Host routing in fp64: per-token top-2 expert ids + softmax gates."""
    n_tokens = x.shape[0]
    logits = x.astype(np.float64) @ Wr.astype(np.float64) + br.astype(np.float64)
    i1 = np.argmax(logits, axis=1)
    ar = np.arange(n_tokens)
    l1 = logits[ar, i1]
    l2m = logits.copy()
    l2m[ar, i1] = -np.inf
    i2 = np.argmax(l2m, axis=1)
    l2 = logits[ar, i2]
    g1 = 1.0 / (1.0 + np.exp(l2 - l1))  # sigmoid(l1 - l2)
    g2 = 1.0 - g1
    return i1, i2, g1.astype(np.float32), g2.astype(np.float32)


def _pack_slots(tiles_needed):
    """Bin-pack per-expert tile counts into the 24 (core,slot) bins.

    Returns {(core, slot): (expert, ntiles)} or None if the greedy fails.
    An expert may span multiple bins (cores); each bin holds one expert.
    """
    sixes = [(c, s) for c in range(N_CORES) for s, cap in enumerate(SLOT_CAPS) if cap == 6]
    fives = [(c, s) for c in range(N_CORES) for s, cap in enumerate(SLOT_CAPS) if cap == 5]
    assign = {}
    for e in sorted(range(N_EXPERTS), key=lambda e: -tiles_needed[e]):
        need = tiles_needed[e]
        while need > 0:
            if need >= 6 and sixes:
                c, s = sixes.pop()
                cap = 6
            elif need <= 5 and fives:
                c, s = fives.pop()
                cap = 5
            elif sixes:
                c, s = sixes.pop()
                cap = 6
            elif fives:
                c, s = fives.pop()
                cap = 5
            else:
                return None
            take = min(need, cap)
            assign[(c, s)] = (e, take)
            need -= take
    return assign


def _prep(inputs):
    x = np.asarray(inputs["x"], dtype=np.float32)
    Wr = np.asarray(inputs["Wr"], dtype=np.float32)
    br = np.asarray(inputs["br"], dtype=np.float32)
    W = np.asarray(inputs["W"], dtype=np.float32)
    b = np.asarray(inputs["b"], dtype=np.float32)
    i1, i2, g1, g2 = _route(x, Wr, br)

    idx_per_e, g_per_e = [], []
    for e in range(N_EXPERTS):
        m1 = i1 == e
        m2 = i2 == e
        idx = np.where(m1 | m2)[0]
        g = np.where(m1[idx], g1[idx], g2[idx])
        idx_per_e.append(idx)
        g_per_e.append(g)

    tiles_needed = [(len(ix) + P - 1) // P for ix in idx_per_e]
    assign = _pack_slots(tiles_needed)
    if assign is not None:
        caps = SLOT_CAPS
    else:
        # fallback: one expert per core, padded to max load
        caps = (max(tiles_needed),)
        assign = {(e, 0): (e, tiles_needed[e]) for e in range(N_EXPERTS)}

    T = sum(caps)
    C = T * P
    base = np.cumsum((0,) + caps[:-1]) * P  # token offset of each slot

    # split each expert's tokens across its bins (in assignment order)
    consumed = [0] * N_EXPERTS
    core_slots = [[] for _ in range(N_CORES)]  # per core: (slot, expert, idx, g)
    for (c, s), (e, take) in sorted(assign.items()):
        lo = consumed[e]
        hi = min(lo + take * P, len(idx_per_e[e]))
        consumed[e] = hi
        core_slots[c].append((s, e, idx_per_e[e][lo:hi], g_per_e[e][lo:hi]))
    for e in range(N_EXPERTS):
        assert consumed[e] == len(idx_per_e[e]), (e, consumed[e], len(idx_per_e[e]))

    in_maps = []
    for c in range(N_CORES):
        xg = np.zeros((C, D), dtype=np.float32)
        wst = np.zeros((len(caps), D, D), dtype=np.float32)
        for s, e, idx, g in core_slots[c]:
            off = base[s]
            xg[off : off + len(idx)] = x[idx] * g[:, None]
            wst[s] = W[e]
        xg_t = np.ascontiguousarray(
            xg.reshape(T, P, KT, P).transpose(3, 0, 2, 1).reshape(P, T * KT * P)
        ).astype(ml_dtypes.bfloat16)
        in_maps.append(
            {
                "xg_t": xg_t,
                "w": np.ascontiguousarray(wst.reshape(len(caps) * D, D)).astype(
                    ml_dtypes.bfloat16
                ),
            }
        )
    return in_maps, core_slots, caps, base, x.shape[0], b


def kernel(**inputs) -> np.ndarray:
    global LAST_RESULTS
    in_maps, core_slots, caps, base, n_tokens, b = _prep(inputs)
    T = sum(caps)
    nc = _build(caps)
    try:
        res = run_bass_kernel_spmd(nc, in_maps, core_ids=list(range(N_CORES)))
    except Exception:  # transient device wedge: back off once and retry
        import time as _time

        _time.sleep(30)
        res = run_bass_kernel_spmd(nc, in_maps, core_ids=list(range(N_CORES)))
    LAST_RESULTS = res

    out = np.zeros((n_tokens, D), dtype=np.float32)
    for c in range(N_CORES):
        # y [P, T*D]: y[p, t*D + f] = token (t*128+p), feature f
        yc = (
            res.results[c]["y"]
            .astype(np.float32)
            .reshape(P, T, D)
            .transpose(1, 0, 2)
            .reshape(T * P, D)
        )
        for s, e, idx, g in core_slots[c]:
            off = base[s]
            out[idx] += yc[off : off + len(idx)] + g[:, None] * b[e][None, :]
    return out
